# revision 4
# baseline (speedup 1.0000x reference)
"""CrossViewAttention3D Trainium2 kernel.

B=1, C=512, T=4, H=32, W=32 -> N=4096 tokens, 8 heads x head_dim 64.
Head-parallel across 8 NeuronCores: core h computes q/k/v projections for
its head, fused flash-style attention, then the Wo column-slice partial
out-projection.  Host sums the 8 partials and adds the output bias.

The pipeline is softmax-exp bound: exp runs only on the Scalar (ACT)
engine at 1 elem/cycle/lane, so the whole kernel is scheduled around
keeping ACT 100% busy with maximal-size activations:
  - S^T tiles land in 3-bank PSUM groups [128,3,512]; one ACTIVATE per
    group (1536 elem/partition) amortizes the ~350-cycle fixed cost.
  - exp writes fp8e4 directly (bias=-3 shifts the exponent range so
    e4m3 never overflows; the shift cancels in the softmax divide).
  - A*V runs as fp8 DoubleRow matmuls (contraction 2x128 tokens per
    instruction at 0.5 cycles/row) so the PE has lots of slack and
    never makes ACT wait.  S matmuls stay fp16, row-packed in pairs via
    tile_position (0,0)/(64,0) with q/k duplicated across partition
    halves (host duplicates the weight columns, so the projections
    produce both copies for free).

Self-contained: hardcodes all shapes; needs numpy + the installed
concourse/bass stack (axon-attached TRN2 cores via jax).
"""
import numpy as np
from collections import deque

import concourse.tile as tile
from concourse import bacc, mybir
from concourse.bass_utils import run_bass_kernel_spmd
from concourse.masks import make_identity

f32 = mybir.dt.float32
MMDT = mybir.dt.float16     # fp16 matmul operand dtype
F8 = mybir.dt.float8e4      # e4m3 for exp(S) and v^T

B, C, T, H, W = 1, 512, 4, 32, 32
NHEADS = 8
D = C // NHEADS          # 64 head dim
P = 128                  # partitions
N = T * H * W            # 4096 tokens
NT = 512                 # n-tile (query block)
NTILES = N // NT         # 8 passes
CCH = C // P             # 4 c-chunks (projection contraction)
MCH = N // P             # 32 key chunks
NPAIR = MCH // 2         # 16 S-pair slots per pass
VPAD = 80                # v1t inner stride (16B-aligned for DoubleRow)
SCALE = float(D) ** -0.5  # 0.125
EXP_SHIFT = -3.0         # exp(s*scale + EXP_SHIFT); cancels in softmax
AV_LAG = 4               # pair slots an AV matmul trails its S pair

_EXP = mybir.ActivationFunctionType.Exp
DR = mybir.MatmulPerfMode.DoubleRow


def _build():
    nc = bacc.Bacc(None, target_bir_lowering=False, debug=False)
    xv = nc.dram_tensor("xv", [C, N], f32, kind="ExternalInput")
    xr = nc.dram_tensor("xr", [C, N], f32, kind="ExternalInput")
    # wq/wk carry the head weight columns duplicated (host sends [C, 2D])
    wq = nc.dram_tensor("wq", [C, 2 * D], f32, kind="ExternalInput")
    wk = nc.dram_tensor("wk", [C, 2 * D], f32, kind="ExternalInput")
    wv = nc.dram_tensor("wv", [C, D], f32, kind="ExternalInput")
    bq = nc.dram_tensor("bq", [2 * D, 1], f32, kind="ExternalInput")
    bk = nc.dram_tensor("bk", [2 * D, 1], f32, kind="ExternalInput")
    bv = nc.dram_tensor("bv", [D, 1], f32, kind="ExternalInput")
    wo = nc.dram_tensor("wo", [D, C], f32, kind="ExternalInput")
    out = nc.dram_tensor("out", [C, N], f32, kind="ExternalOutput")

    xv_r = xv.rearrange("(o p) n -> p o n", p=P)
    xr_r = xr.rearrange("(o p) n -> p o n", p=P)

    with tile.TileContext(nc) as tc:
        with (
            tc.tile_pool(name="const", bufs=1) as const,
            tc.tile_pool(name="persist", bufs=1) as persist,
            tc.tile_pool(name="pfull", bufs=2) as pfull,
            tc.tile_pool(name="xload", bufs=3) as xload,
            tc.tile_pool(name="stage", bufs=4) as stage,
        ):
            # ---- weights / biases / identity ----
            wq_sb = const.tile([P, CCH, 2 * D], MMDT, tag="wq")
            wk_sb = const.tile([P, CCH, 2 * D], MMDT, tag="wk")
            wv_sb = const.tile([P, CCH, D], MMDT, tag="wv")
            nc.gpsimd.dma_start(wq_sb[:], wq.rearrange("(o p) m -> p o m", p=P))
            nc.gpsimd.dma_start(wk_sb[:], wk.rearrange("(o p) m -> p o m", p=P))
            nc.gpsimd.dma_start(wv_sb[:], wv.rearrange("(o p) m -> p o m", p=P))
            wo_sb = const.tile([D, C], MMDT, tag="wo")
            nc.gpsimd.dma_start(wo_sb[:], wo[:])
            bq_sb = const.tile([2 * D, 1], f32, tag="bq")
            bk_sb = const.tile([2 * D, 1], f32, tag="bk")
            bv_sb = const.tile([D, 1], f32, tag="bv")
            nc.sync.dma_start(bq_sb[:], bq[:])
            nc.sync.dma_start(bk_sb[:], bk[:])
            nc.sync.dma_start(bv_sb[:], bv[:])
            nbias = const.tile([P, 1], f32, tag="nbias")
            nc.vector.memset(nbias[:], EXP_SHIFT)

            ident = const.tile([D, D], MMDT, tag="ident")
            make_identity(nc, ident[:])
            # ---- persistent activations ----
            q_sb = persist.tile([P, N], MMDT, tag="q")    # rows 64:128 dup
            k_sb = persist.tile([P, N], MMDT, tag="k")
            v_sb = persist.tile([D, N], MMDT, tag="v")
            # v^T in fp8, padded inner stride, col D holds the ones column
            v1t = persist.tile([P, MCH, VPAD], F8, tag="v1t")
            ones8 = const.tile([P, MCH], F8, tag="ones8")
            nc.vector.memset(ones8[:], 1.0)
            nc.vector.tensor_copy(v1t[:, :, D], ones8[:])

            with (
                tc.tile_pool(name="ps_s", bufs=2, space="PSUM") as ps_s,
                tc.tile_pool(name="ps_o", bufs=1, space="PSUM") as ps_o,
                tc.tile_pool(name="ps_op", bufs=1, space="PSUM") as ps_op,
            ):
                # ---- projection helpers ----
                def load_x(dram_r, nt, tag):
                    ns = slice(nt * NT, (nt + 1) * NT)
                    raw = xload.tile([P, CCH, NT], f32, tag=tag + "r",
                                     name=f"{tag}r_{nt}")
                    x16 = xload.tile([P, CCH, NT], MMDT, tag=tag,
                                     name=f"{tag}_{nt}")
                    for cc in range(CCH):
                        nc.sync.dma_start(raw[:, cc], dram_r[:, cc, ns])
                        nc.vector.tensor_copy(x16[:, cc], raw[:, cc])
                    return x16

                def proj(dst, w_sb, b_sb, x16, nt, rows):
                    ns = slice(nt * NT, (nt + 1) * NT)
                    ps = ps_op.tile([P, NT], f32, tag="op", name=f"pj_{nt}")
                    for cc in range(CCH):
                        nc.tensor.matmul(ps[:rows], w_sb[:, cc], x16[:, cc],
                                         start=(cc == 0), stop=(cc == CCH - 1))
                    nc.vector.tensor_add(dst[:, ns], ps[:rows],
                                         b_sb[:, 0:1].to_broadcast([rows, NT]))

                def q_proj(nt):
                    x16 = load_x(xv_r, nt, "xv")
                    proj(q_sb, wq_sb, bq_sb, x16, nt, P)

                # ---- scheduler state ----
                s_tiles = {}
                p_tiles = {}
                o_tiles = {}
                s_slot = {}         # (nt, j) -> emit slot of that S pair
                av_cur = [0, 0]     # next AV to emit, in pass-major order
                pendings = deque()
                slot = [0]

                def emit_chunk_s(nt, c):
                    gi = c // 3
                    if (nt, gi) not in s_tiles:
                        s_tiles[(nt, gi)] = ps_s.tile(
                            [P, 3, NT], f32, tag="s", name=f"s_{nt}_{gi}")
                    t = s_tiles[(nt, gi)]
                    ns = slice(nt * NT, (nt + 1) * NT)
                    if c % 2 == 0:
                        nc.tensor.matmul(
                            t[:, c - 3 * gi], k_sb[0:D, c * P:(c + 1) * P],
                            q_sb[0:D, ns], start=True, stop=True,
                            tile_position=(0, 0))
                    else:
                        nc.tensor.matmul(
                            t[:, c - 3 * gi], k_sb[D:P, c * P:(c + 1) * P],
                            q_sb[D:P, ns], start=True, stop=True,
                            tile_position=(64, 0))
                    if c == min(3 * gi + 2, MCH - 1):
                        ch = min(3, MCH - 3 * gi)
                        nc.scalar.activation(
                            p_tiles[nt][:, 3 * gi:3 * gi + ch, :],
                            t[:, 0:ch], _EXP, scale=SCALE, bias=nbias[:])
                        s_tiles.pop((nt, gi))

                def emit_av(nt, j):
                    if j == 0:
                        o_tiles[nt] = ps_o.tile([P, NT], f32, tag="o",
                                                name=f"o_{nt}")
                    nc.tensor.matmul(
                        o_tiles[nt][:D + 1], v1t[:, 2 * j:2 * j + 2, 0:D + 1],
                        p_tiles[nt][:, 2 * j:2 * j + 2, :],
                        start=(j == 0), stop=(j == NPAIR - 1), perf_mode=DR)
                    if j == NPAIR - 1:
                        pendings.append([nt, *epilogue_head(nt)])

                def drain(force=False):
                    # AVs go out in strict pass-major order: pass nt's o-bank
                    # (single PSUM buf) must fully retire before pass nt+1's
                    # first AV hits the PE queue, or the queue deadlocks.
                    while True:
                        nt0, j0 = av_cur
                        if (nt0, j0) not in s_slot:
                            break
                        if not force and s_slot[(nt0, j0)] + AV_LAG > slot[0]:
                            break
                        emit_av(nt0, j0)
                        if j0 == NPAIR - 1:
                            av_cur[0], av_cur[1] = nt0 + 1, 0
                        else:
                            av_cur[1] = j0 + 1

                def push_s(nt, j):
                    if j == 0:
                        p_tiles[nt] = pfull.tile([P, MCH, NT], F8, tag="pf",
                                                 name=f"pf_{nt}")
                    emit_chunk_s(nt, 2 * j)
                    emit_chunk_s(nt, 2 * j + 1)
                    s_slot[(nt, j)] = slot[0]
                    slot[0] += 1
                    drain()

                def epilogue_head(nt):
                    # normalization by the softmax denominator commutes with
                    # the Wo matmul: out-projection consumes UNNORMALIZED O,
                    # the divide happens on the projected tiles in the tail.
                    o_ps = o_tiles.pop(nt)
                    obar16 = stage.tile([D, NT], MMDT, tag="obar", bufs=2)
                    nc.vector.tensor_copy(obar16[:], o_ps[:D])
                    den = stage.tile([1, NT], f32, tag="den", bufs=2)
                    nc.vector.tensor_copy(den[:], o_ps[D:D + 1])
                    rec = stage.tile([1, NT], f32, tag="rec", bufs=2)
                    rscr = stage.tile([1, NT], f32, tag="rscr", bufs=2)
                    nc.vector.reciprocal_approx_accurate(rec[:], den[:],
                                                         rscr[:])
                    rb = stage.tile([P, NT], f32, tag="rb", bufs=2)
                    nc.gpsimd.partition_broadcast(rb[:], rec[:])
                    return obar16, rb

                def epilogue_tail(nt, obar16, rb):
                    ns = slice(nt * NT, (nt + 1) * NT)
                    for cc in range(CCH):
                        op_ps = ps_op.tile([P, NT], f32, tag="op",
                                           name=f"opj_{nt}_{cc}")
                        nc.tensor.matmul(op_ps[:],
                                         wo_sb[:, cc * P:(cc + 1) * P],
                                         obar16[:], start=True, stop=True)
                        ot = stage.tile([P, NT], f32, tag="ot")
                        nc.vector.tensor_mul(ot[:], op_ps[:], rb[:])
                        nc.sync.dma_start(out[cc * P:(cc + 1) * P, ns], ot[:])

                # ---- interleaved prologue + passes 0 and 1 ----
                # group g: load xr tile g, project k/v, transpose v chunks;
                # pass-0/1 S-pairs slot in behind the k/v1t chunks they need
                # so ACT starts filling while the prologue is still streaming.
                for g in range(NTILES):
                    x16 = load_x(xr_r, g, "xr")
                    proj(k_sb, wk_sb, bk_sb, x16, g, P)
                    proj(v_sb, wv_sb, bv_sb, x16, g, D)
                    vt_ps = ps_op.tile([P, 4, D], MMDT, tag="op",
                                       name=f"vt_{g}")
                    for i, mc in enumerate(range(4 * g, 4 * g + 4)):
                        nc.tensor.transpose(
                            vt_ps[:, i], v_sb[:, mc * P:(mc + 1) * P],
                            ident[:])
                    nc.vector.tensor_copy(
                        v1t[:, 4 * g:4 * g + 4, 0:D], vt_ps[:])
                    if g == 0:
                        q_proj(0)
                        push_s(0, 0)
                        push_s(0, 1)
                    elif g == 1:
                        push_s(0, 2)
                        push_s(0, 3)
                        q_proj(1)
                    else:
                        push_s(0, 2 * g)
                        push_s(0, 2 * g + 1)
                        push_s(1, 2 * (g - 2))
                        push_s(1, 2 * (g - 2) + 1)
                q_proj(2)
                for j in range(2 * (NTILES - 2), NPAIR):
                    push_s(1, j)

                # ---- remaining passes ----
                for nt in range(2, NTILES):
                    for j in range(NPAIR):
                        push_s(nt, j)
                        if j in (4, 12) and pendings:
                            epilogue_tail(*pendings.popleft())
                        if j == 8 and nt + 1 < NTILES:
                            q_proj(nt + 1)
                drain(force=True)
                while pendings:
                    epilogue_tail(*pendings.popleft())
    nc.compile()
    return nc


_cached_nc = None


def _get_nc():
    global _cached_nc
    if _cached_nc is None:
        _cached_nc = _build()
    return _cached_nc


def _make_in_maps(inp):
    xv = np.ascontiguousarray(inp["video_feat"].reshape(C, N), dtype=np.float32)
    xr = np.ascontiguousarray(inp["ref_feat"].reshape(C, N), dtype=np.float32)

    def dupc(a):  # duplicate columns: [C, D] -> [C, 2D]
        return np.ascontiguousarray(np.concatenate([a, a], axis=1),
                                    dtype=np.float32)

    in_maps = []
    for h in range(NHEADS):
        sl = slice(h * D, (h + 1) * D)
        wq_t = inp["Wq"][sl].T
        wk_t = inp["Wk"][sl].T
        in_maps.append({
            "xv": xv,
            "xr": xr,
            "wq": dupc(wq_t),
            "wk": dupc(wk_t),
            "wv": np.ascontiguousarray(inp["Wv"][sl].T, dtype=np.float32),
            "bq": np.ascontiguousarray(
                np.tile(inp["bq"][sl], 2).reshape(2 * D, 1), dtype=np.float32),
            "bk": np.ascontiguousarray(
                np.tile(inp["bk"][sl], 2).reshape(2 * D, 1), dtype=np.float32),
            "bv": np.ascontiguousarray(
                inp["bv"][sl].reshape(D, 1), dtype=np.float32),
            "wo": np.ascontiguousarray(inp["Wo"][:, sl].T, dtype=np.float32),
        })
    return in_maps


def run(inputs, **spmd_kwargs):
    """Run the kernel; returns (full_output, BassKernelResults)."""
    inp = {k: np.asarray(v) for k, v in inputs.items()}
    nc = _get_nc()
    res = run_bass_kernel_spmd(nc, _make_in_maps(inp),
                               list(range(NHEADS)), **spmd_kwargs)
    total = res.results[0]["out"].astype(np.float32).copy()
    for r in res.results[1:]:
        total += r["out"]
    total += np.asarray(inp["bo"], dtype=np.float32)[:, None]
    return total.reshape(B, C, T, H, W), res


def kernel(**inputs):
    out, _ = run(inputs)
    return out
